# revision 28
# baseline (speedup 1.0000x reference)
"""Trainium2 Bass kernel for nn_CedrDrmmRanker (histogram_binning).

Computation (per layer l, batch b over hidden_states [13,16,512,768] f32):
  sim[q,d] = cos(x_q, x_d) for q in first 20 tokens, d in remaining 492
  hist     = 11-bin histogram of sim over [-1,1]
  hfeat    = hist @ W_hist.T + b_hist
  out[b]   = concat(cls, hfeat-all-layers) @ W_comb.T + b_comb

Device strategy (pure data parallel, batch sharded 2-per-core over 8 cores):
  Host precomputes the inverse-norm outer product dmi[q,s] = 1/(|x_q||x_s|)
  (tiny: [26,20,512] f32 per core).  Per (l,b) pair on device: cast-DMA
  fp32->bf16 (SWDGE on Pool), PE identity-matmul transpose to [h,s] layout
  (PSUM->SBUF copies split DVE/ACT), PE Gram matmul q x all-tokens, DVE
  normalize via dmi + is_ge boundary counts.  The PE stream is software-
  pipelined one pair deep (transposes of pair p, then Gram of pair p-1) so
  no engine queue blocks on a cross-engine dependency.  Device emits
  per-(pair,q-row) >=boundary counts; the tiny histogram/linear algebra
  runs on host in fp32.
"""

import os
import sys

import numpy as np

for _p in ("/opt/trn_rl_repo",):
    if os.path.isdir(_p) and _p not in sys.path:
        sys.path.append(_p)

# ---- problem constants (hardcoded; kernel.py must be self-contained) ----
L = 13          # layers
B = 16          # global batch
S = 512         # sequence
H = 768         # hidden
NQ = 20         # query tokens
ND = S - NQ     # 492 doc tokens
N_BINS = 11
N_CORES = 8
BC = B // N_CORES          # 2 batches per core
PAIRS = L * BC             # 26 (layer-major: p = l*BC + b)
GSIZE = 4                  # pairs per count-group (32-partition slots)
NGROUPS = (PAIRS + GSIZE - 1) // GSIZE   # 7
NB = 10                    # interior boundaries b1..b10 counted on device
SCH = S // 128             # 4 S-chunks
HCH = H // 128             # 6 H-chunks

_BOUNDS = np.linspace(-1.0, 1.0, N_BINS + 1).astype(np.float32)  # 12 boundaries


def _build_nc(npairs=PAIRS, num_devices=N_CORES, nreps=1):
    import concourse.bass as bass
    import concourse.tile as tile
    from concourse import bacc, mybir
    from concourse.masks import make_identity
    from contextlib import ExitStack

    f32 = mybir.dt.float32
    bf16 = mybir.dt.bfloat16

    nc = bacc.Bacc(
        "TRN2",
        target_bir_lowering=False,
        debug=False,
        num_devices=num_devices,
        dynamic_dma_scratch_size=65536,
    )
    hs = nc.dram_tensor("hs", [L, BC, S, H], f32, kind="ExternalInput").ap()
    dmi = nc.dram_tensor("dmi", [L, BC, NQ, S], bf16, kind="ExternalInput").ap()
    counts = nc.dram_tensor(
        "counts", [NGROUPS, 128, NB], f32, kind="ExternalOutput"
    ).ap()

    with tile.TileContext(nc) as tc, ExitStack() as ctx:
        consts = ctx.enter_context(tc.tile_pool(name="consts", bufs=1))
        xpool = ctx.enter_context(tc.tile_pool(name="x", bufs=5))
        xtpool = ctx.enter_context(tc.tile_pool(name="xt", bufs=3))
        dmipool = ctx.enter_context(tc.tile_pool(name="dmi", bufs=4))
        sqpool = ctx.enter_context(tc.tile_pool(name="sq", bufs=4))
        gpool = ctx.enter_context(tc.tile_pool(name="grp", bufs=3))
        psA = ctx.enter_context(tc.tile_pool(name="psA", bufs=3, space="PSUM"))
        psB = ctx.enter_context(tc.tile_pool(name="psB", bufs=3, space="PSUM"))

        ident_bf = consts.tile([128, 128], bf16, tag="identb")
        make_identity(nc, ident_bf[:])

        mult = mybir.AluOpType.mult
        bypass = mybir.AluOpType.bypass
        is_ge = mybir.AluOpType.is_ge

        rep_ctx = tc.For_i(0, nreps, 1) if nreps > 1 else None
        if rep_ctx is not None:
            ctx.enter_context(rep_ctx)

        def emit_stage_a(p):
            """Load + transpose pair p; returns tiles needed later."""
            g, i = divmod(p, GSIZE)
            l, b = divmod(p, BC)

            if i == 0:
                # new count group: park unused rows below -1 (DVE, ahead of
                # this group's sim writes in the DVE queue)
                simgrp = gpool.tile([128, ND], bf16, tag="sim")
                nc.vector.memset(simgrp[:], -2.0)
                emit_stage_a.simgrp = simgrp
            simgrp = emit_stage_a.simgrp

            # normalization matrix (host-computed), dep-free tiny DMA on SP
            dmi_sb = dmipool.tile([NQ, S], bf16, tag="dmi")
            nc.sync.dma_start(dmi_sb[:], dmi[l, b])

            # load + cast fp32 -> bf16   [128, 4, 768]  (SWDGE on Pool)
            # partition p holds tokens 4p..4p+3: one contiguous 12 KiB DRAM
            # run per partition (128 descriptors instead of 512)
            xb = xpool.tile([128, SCH, H], bf16, tag="xb")
            nc.gpsimd.dma_start(
                xb[:], hs[l, b].rearrange("(p t) h -> p t h", t=SCH)
            )

            # PE identity transposes -> xt [128, 6, 512]  (copies DVE 1/3,
            # ACT 2/3).  The PSUM column order is (t, i) <-> token 4i+t;
            # the copy writes a strided destination so xt holds natural
            # token order and the Gram operands stay contiguous.
            xt = xtpool.tile([128, HCH, S], bf16, tag="xt")
            for j in range(3):  # 2 h-chunks per PSUM tile
                xtps = psA.tile([128, 2 * S], bf16, tag="xtps")
                for u in range(2):
                    h = 2 * j + u
                    for t in range(SCH):
                        nc.tensor.transpose(
                            xtps[:, u * S + t * 128 : u * S + (t + 1) * 128],
                            xb[:, t, h * 128 : (h + 1) * 128],
                            ident_bf[:],
                        )
                # dst col for PSUM col (u, t, i) is (u, i*4+t)
                xt_dst = (
                    xt[:, 2 * j : 2 * j + 2, :]
                    .rearrange("p a (i t) -> p a t i", t=SCH)
                )
                xt_src = xtps[:].rearrange("p (a t i) -> p a t i", a=2, t=SCH)
                if j == 0:
                    nc.vector.tensor_copy(out=xt_dst, in_=xt_src)
                else:
                    nc.scalar.copy(out=xt_dst, in_=xt_src)
            return (p, simgrp, dmi_sb, xt)

        def emit_stage_b(st):
            """Gram + normalize + (maybe) counts for a pair from stage a."""
            p, simgrp, dmi_sb, xt = st
            g, i = divmod(p, GSIZE)
            r0 = 32 * i

            dots_ps = psB.tile([128, S], f32, tag="dots")
            for hc in range(HCH):
                nc.tensor.matmul(
                    dots_ps[0:NQ, :],
                    lhsT=xt[:, hc, 0:NQ],
                    rhs=xt[:, hc, :],
                    start=(hc == 0),
                    stop=(hc == HCH - 1),
                )

            # sim = dots * dmi  (skip q-q cols) -> bf16 group tile
            nc.vector.scalar_tensor_tensor(
                out=simgrp[r0 : r0 + NQ, :],
                in0=dots_ps[0:NQ, NQ:S],
                scalar=0.0,
                in1=dmi_sb[:, NQ:S],
                op0=bypass, op1=mult,
            )

            if i == GSIZE - 1 or p == npairs - 1:
                # boundary counts: cnt[:, k] = sum_d (sim >= b_k) on DVE
                cntg = gpool.tile([128, NB], f32, tag="cnt")
                for k in range(NB):
                    csc = sqpool.tile([128, ND], bf16, tag="csc")
                    nc.vector.tensor_scalar(
                        out=csc[:], in0=simgrp[:],
                        scalar1=float(_BOUNDS[k + 1]), scalar2=None,
                        op0=is_ge, op1=mybir.AluOpType.add,
                        accum_out=cntg[:, k : k + 1],
                    )
                nc.scalar.dma_start(counts[g], cntg[:])

        pending = None
        for p in range(npairs):
            st = emit_stage_a(p)
            if pending is not None:
                emit_stage_b(pending)
            pending = st
        emit_stage_b(pending)

    nc.compile()
    return nc


_NC_CACHE = None


def _get_nc():
    global _NC_CACHE
    if _NC_CACHE is None:
        _NC_CACHE = _build_nc()
    return _NC_CACHE


def _dmi_host(hs):
    """Host: dmi[l,b,q,s] = 1/(|x_q| |x_s|), shipped as bf16."""
    import ml_dtypes

    n2 = np.einsum("lbsh,lbsh->lbs", hs, hs, dtype=np.float32)
    inv = 1.0 / np.sqrt(np.maximum(n2, 1e-16))
    return (inv[:, :, :NQ, None] * inv[:, :, None, :]).astype(ml_dtypes.bfloat16)


def _postprocess(counts_per_core, hidden_states, W_hist, b_hist, W_comb, b_comb):
    """counts_per_core: list of 8 arrays [NGROUPS, 128, NB]."""
    hs = np.asarray(hidden_states, dtype=np.float32)
    W_hist = np.asarray(W_hist, np.float32)
    b_hist = np.asarray(b_hist, np.float32)
    W_comb = np.asarray(W_comb, np.float32)
    b_comb = np.asarray(b_comb, np.float32)

    # N_ge counts per (core, pair, boundary)
    hist = np.zeros((L, B, N_BINS), np.float32)
    total = float(NQ * ND)
    for c in range(N_CORES):
        cc = counts_per_core[c]  # [NGROUPS, 128, NB]
        for p in range(PAIRS):
            g, i = divmod(p, GSIZE)
            l, bl = divmod(p, BC)
            n_ge = cc[g, 32 * i : 32 * i + NQ, :].sum(axis=0)  # [NB]
            n_full = np.empty(N_BINS + 1, np.float64)
            n_full[0] = total
            n_full[1 : NB + 1] = n_ge
            n_full[N_BINS] = 0.0
            hist[l, c * BC + bl] = (n_full[:-1] - n_full[1:]) / total

    # histogram features for the 14 "all_layers" (layer 0 duplicated)
    hist14 = np.concatenate([hist[:1], hist], axis=0)  # [14, B, 11]
    hfeat = hist14 @ W_hist.T + b_hist  # [14, B, 5]
    histogram_features = np.transpose(hfeat, (1, 0, 2)).reshape(B, -1)  # [B, 70]

    cls_output = hs[-1][:, 0, :]  # [B, H]
    combined = np.concatenate([cls_output, histogram_features], axis=-1)
    return (combined @ W_comb.T + b_comb).astype(np.float32)  # [B, 1]


def _in_maps(hs):
    """Per-core input maps (hs shard + host-computed dmi)."""
    dmi = _dmi_host(hs)
    return [
        {
            "hs": np.ascontiguousarray(hs[:, c * BC : (c + 1) * BC]),
            "dmi": np.ascontiguousarray(dmi[:, c * BC : (c + 1) * BC]),
        }
        for c in range(N_CORES)
    ]


def kernel(hidden_states, W_hist, b_hist, W_comb, b_comb):
    from concourse.bass_utils import run_bass_kernel_spmd

    nc = _get_nc()
    hs = np.ascontiguousarray(np.asarray(hidden_states, dtype=np.float32))
    res = run_bass_kernel_spmd(nc, _in_maps(hs), core_ids=list(range(N_CORES)))
    counts_per_core = [np.asarray(res.results[c]["counts"]) for c in range(N_CORES)]
    return _postprocess(
        counts_per_core, hidden_states, W_hist, b_hist, W_comb, b_comb
    )


# revision 29
# speedup vs baseline: 1.0745x; 1.0745x over previous
"""Trainium2 Bass kernel for nn_CedrDrmmRanker (histogram_binning).

Computation (per layer l, batch b over hidden_states [13,16,512,768] f32):
  sim[q,d] = cos(x_q, x_d) for q in first 20 tokens, d in remaining 492
  hist     = 11-bin histogram of sim over [-1,1]
  hfeat    = hist @ W_hist.T + b_hist
  out[b]   = concat(cls, hfeat-all-layers) @ W_comb.T + b_comb

Device strategy (pure data parallel, batch sharded 2-per-core over 8 cores):
  Host precomputes the inverse-norm outer product dmi[q,s] = 1/(|x_q||x_s|)
  (tiny: [26,20,512] f32 per core).  Per (l,b) pair on device: cast-DMA
  fp32->bf16 (SWDGE on Pool), PE identity-matmul transpose to [h,s] layout
  (PSUM->SBUF copies split DVE/ACT), PE Gram matmul q x all-tokens, DVE
  normalize via dmi + is_ge boundary counts.  The PE stream is software-
  pipelined one pair deep (transposes of pair p, then Gram of pair p-1) so
  no engine queue blocks on a cross-engine dependency.  Device emits
  per-(pair,q-row) >=boundary counts; the tiny histogram/linear algebra
  runs on host in fp32.
"""

import os
import sys

import numpy as np

for _p in ("/opt/trn_rl_repo",):
    if os.path.isdir(_p) and _p not in sys.path:
        sys.path.append(_p)

# ---- problem constants (hardcoded; kernel.py must be self-contained) ----
L = 13          # layers
B = 16          # global batch
S = 512         # sequence
H = 768         # hidden
NQ = 20         # query tokens
ND = S - NQ     # 492 doc tokens
N_BINS = 11
N_CORES = 8
BC = B // N_CORES          # 2 batches per core
PAIRS = L * BC             # 26 (layer-major: p = l*BC + b)
GSIZE = 4                  # pairs per count-group (32-partition slots)
NGROUPS = (PAIRS + GSIZE - 1) // GSIZE   # 7
NB = 10                    # interior boundaries b1..b10 counted on device
SCH = S // 128             # 4 S-chunks
HCH = H // 128             # 6 H-chunks

_BOUNDS = np.linspace(-1.0, 1.0, N_BINS + 1).astype(np.float32)  # 12 boundaries


def _build_nc(npairs=PAIRS, num_devices=N_CORES, nreps=1):
    import concourse.bass as bass
    import concourse.tile as tile
    from concourse import bacc, mybir
    from concourse.masks import make_identity
    from contextlib import ExitStack

    f32 = mybir.dt.float32
    bf16 = mybir.dt.bfloat16

    nc = bacc.Bacc(
        "TRN2",
        target_bir_lowering=False,
        debug=False,
        num_devices=num_devices,
        dynamic_dma_scratch_size=65536,
    )
    hs = nc.dram_tensor("hs", [L, BC, S, H], f32, kind="ExternalInput").ap()
    dmi = nc.dram_tensor("dmi", [L, BC, NQ, S], f32, kind="ExternalInput").ap()
    counts = nc.dram_tensor(
        "counts", [NGROUPS, 128, NB], f32, kind="ExternalOutput"
    ).ap()

    with tile.TileContext(nc) as tc, ExitStack() as ctx:
        consts = ctx.enter_context(tc.tile_pool(name="consts", bufs=1))
        xpool = ctx.enter_context(tc.tile_pool(name="x", bufs=5))
        xtpool = ctx.enter_context(tc.tile_pool(name="xt", bufs=3))
        dmipool = ctx.enter_context(tc.tile_pool(name="dmi", bufs=4))
        sqpool = ctx.enter_context(tc.tile_pool(name="sq", bufs=4))
        gpool = ctx.enter_context(tc.tile_pool(name="grp", bufs=3))
        psA = ctx.enter_context(tc.tile_pool(name="psA", bufs=3, space="PSUM"))
        psB = ctx.enter_context(tc.tile_pool(name="psB", bufs=3, space="PSUM"))

        ident_bf = consts.tile([128, 128], bf16, tag="identb")
        make_identity(nc, ident_bf[:])

        mult = mybir.AluOpType.mult
        bypass = mybir.AluOpType.bypass
        is_ge = mybir.AluOpType.is_ge

        rep_ctx = tc.For_i(0, nreps, 1) if nreps > 1 else None
        if rep_ctx is not None:
            ctx.enter_context(rep_ctx)

        def emit_stage_a(p):
            """Load + transpose pair p; returns tiles needed later."""
            g, i = divmod(p, GSIZE)
            l, b = divmod(p, BC)

            if i == 0:
                # new count group: park unused rows below -1 (DVE, ahead of
                # this group's sim writes in the DVE queue)
                simgrp = gpool.tile([128, ND], bf16, tag="sim")
                nc.vector.memset(simgrp[:], -2.0)
                emit_stage_a.simgrp = simgrp
            simgrp = emit_stage_a.simgrp

            # normalization matrix (host-computed), dep-free tiny DMA on SP
            dmi_sb = dmipool.tile([NQ, S], f32, tag="dmi")
            nc.sync.dma_start(dmi_sb[:], dmi[l, b])

            # load + cast fp32 -> bf16   [128, 4, 768]  (SWDGE on Pool)
            # partition p holds tokens 4p..4p+3: one contiguous 12 KiB DRAM
            # run per partition (128 descriptors instead of 512)
            xb = xpool.tile([128, SCH, H], bf16, tag="xb")
            nc.gpsimd.dma_start(
                xb[:], hs[l, b].rearrange("(p t) h -> p t h", t=SCH)
            )

            # PE identity transposes -> xt [128, 6, 512]  (copies DVE 1/3,
            # ACT 2/3).  The PSUM column order is (t, i) <-> token 4i+t;
            # the copy writes a strided destination so xt holds natural
            # token order and the Gram operands stay contiguous.
            xt = xtpool.tile([128, HCH, S], bf16, tag="xt")
            for j in range(3):  # 2 h-chunks per PSUM tile
                xtps = psA.tile([128, 2 * S], bf16, tag="xtps")
                for u in range(2):
                    h = 2 * j + u
                    for t in range(SCH):
                        nc.tensor.transpose(
                            xtps[:, u * S + t * 128 : u * S + (t + 1) * 128],
                            xb[:, t, h * 128 : (h + 1) * 128],
                            ident_bf[:],
                        )
                # dst col for PSUM col (u, t, i) is (u, i*4+t)
                xt_dst = (
                    xt[:, 2 * j : 2 * j + 2, :]
                    .rearrange("p a (i t) -> p a t i", t=SCH)
                )
                xt_src = xtps[:].rearrange("p (a t i) -> p a t i", a=2, t=SCH)
                if j == 0:
                    nc.vector.tensor_copy(out=xt_dst, in_=xt_src)
                else:
                    nc.scalar.copy(out=xt_dst, in_=xt_src)
            return (p, simgrp, dmi_sb, xt)

        def emit_stage_b(st):
            """Gram + normalize + (maybe) counts for a pair from stage a."""
            p, simgrp, dmi_sb, xt = st
            g, i = divmod(p, GSIZE)
            r0 = 32 * i

            dots_ps = psB.tile([128, S], f32, tag="dots")
            for hc in range(HCH):
                nc.tensor.matmul(
                    dots_ps[0:NQ, :],
                    lhsT=xt[:, hc, 0:NQ],
                    rhs=xt[:, hc, :],
                    start=(hc == 0),
                    stop=(hc == HCH - 1),
                )

            # sim = dots * dmi  (skip q-q cols) -> bf16 group tile
            nc.vector.scalar_tensor_tensor(
                out=simgrp[r0 : r0 + NQ, :],
                in0=dots_ps[0:NQ, NQ:S],
                scalar=0.0,
                in1=dmi_sb[:, NQ:S],
                op0=bypass, op1=mult,
            )

            if i == GSIZE - 1 or p == npairs - 1:
                # boundary counts: cnt[:, k] = sum_d (sim >= b_k) on DVE
                cntg = gpool.tile([128, NB], f32, tag="cnt")
                for k in range(NB):
                    csc = sqpool.tile([128, ND], bf16, tag="csc")
                    nc.vector.tensor_scalar(
                        out=csc[:], in0=simgrp[:],
                        scalar1=float(_BOUNDS[k + 1]), scalar2=None,
                        op0=is_ge, op1=mybir.AluOpType.add,
                        accum_out=cntg[:, k : k + 1],
                    )
                nc.scalar.dma_start(counts[g], cntg[:])

        pending = None
        for p in range(npairs):
            st = emit_stage_a(p)
            if pending is not None:
                emit_stage_b(pending)
            pending = st
        emit_stage_b(pending)

    nc.compile()
    return nc


_NC_CACHE = None


def _get_nc():
    global _NC_CACHE
    if _NC_CACHE is None:
        _NC_CACHE = _build_nc()
    return _NC_CACHE


def _dmi_host(hs):
    """Host: dmi[l,b,q,s] = 1/(|x_q| |x_s|) in fp32."""
    n2 = np.einsum("lbsh,lbsh->lbs", hs, hs, dtype=np.float32)
    inv = 1.0 / np.sqrt(np.maximum(n2, 1e-16))
    return (inv[:, :, :NQ, None] * inv[:, :, None, :]).astype(np.float32)


def _postprocess(counts_per_core, hidden_states, W_hist, b_hist, W_comb, b_comb):
    """counts_per_core: list of 8 arrays [NGROUPS, 128, NB]."""
    hs = np.asarray(hidden_states, dtype=np.float32)
    W_hist = np.asarray(W_hist, np.float32)
    b_hist = np.asarray(b_hist, np.float32)
    W_comb = np.asarray(W_comb, np.float32)
    b_comb = np.asarray(b_comb, np.float32)

    # N_ge counts per (core, pair, boundary)
    hist = np.zeros((L, B, N_BINS), np.float32)
    total = float(NQ * ND)
    for c in range(N_CORES):
        cc = counts_per_core[c]  # [NGROUPS, 128, NB]
        for p in range(PAIRS):
            g, i = divmod(p, GSIZE)
            l, bl = divmod(p, BC)
            n_ge = cc[g, 32 * i : 32 * i + NQ, :].sum(axis=0)  # [NB]
            n_full = np.empty(N_BINS + 1, np.float64)
            n_full[0] = total
            n_full[1 : NB + 1] = n_ge
            n_full[N_BINS] = 0.0
            hist[l, c * BC + bl] = (n_full[:-1] - n_full[1:]) / total

    # histogram features for the 14 "all_layers" (layer 0 duplicated)
    hist14 = np.concatenate([hist[:1], hist], axis=0)  # [14, B, 11]
    hfeat = hist14 @ W_hist.T + b_hist  # [14, B, 5]
    histogram_features = np.transpose(hfeat, (1, 0, 2)).reshape(B, -1)  # [B, 70]

    cls_output = hs[-1][:, 0, :]  # [B, H]
    combined = np.concatenate([cls_output, histogram_features], axis=-1)
    return (combined @ W_comb.T + b_comb).astype(np.float32)  # [B, 1]


def _in_maps(hs):
    """Per-core input maps (hs shard + host-computed dmi)."""
    dmi = _dmi_host(hs)
    return [
        {
            "hs": np.ascontiguousarray(hs[:, c * BC : (c + 1) * BC]),
            "dmi": np.ascontiguousarray(dmi[:, c * BC : (c + 1) * BC]),
        }
        for c in range(N_CORES)
    ]


def kernel(hidden_states, W_hist, b_hist, W_comb, b_comb):
    from concourse.bass_utils import run_bass_kernel_spmd

    nc = _get_nc()
    hs = np.ascontiguousarray(np.asarray(hidden_states, dtype=np.float32))
    res = run_bass_kernel_spmd(nc, _in_maps(hs), core_ids=list(range(N_CORES)))
    counts_per_core = [np.asarray(res.results[c]["counts"]) for c in range(N_CORES)]
    return _postprocess(
        counts_per_core, hidden_states, W_hist, b_hist, W_comb, b_comb
    )
